# revision 6
# baseline (speedup 1.0000x reference)
"""GT layer (graph transformer message passing) on 8 TRN2 NeuronCores.

N=100000 nodes, E=800000 edges, D=64, H=4 heads.

Strategy (SPMD, one identical program on 8 cores):
- Nodes are permuted by in-degree (descending) and dealt round-robin to the 8
  cores, so every core owns 12544 destination nodes with a near-identical
  degree profile -> identical per-tile chunk counts -> one shared program.
- Destination nodes map to (tile, partition): tile ti holds 128 nodes, one per
  SBUF partition. Edge k of a node sits at chunk k on that partition
  ("identity layout"): the segment softmax + scatter-add become pure
  per-partition reductions over the chunk axis - no one-hot matmuls.
- The host performs the index gather (transposed: xcT[d, slot] = emb[col, d]
  as fp16), because this environment's SWDGE gather primitives are unusable
  (dma_gather fails walrus codegen; indirect_dma_start costs ~1.9us/128 rows).
  The device does all FLOPs: q/k/v projections (PE), attention scores +
  clip + exp (DVE/ACT), masking, weighted value aggregation and the segment
  reductions (DVE), final normalization (DVE).
- Outputs come back as [12544, 64] per core and are un-permuted on the host.
"""
import numpy as np

N = 100000
NP = 100352          # 784 tiles of 128
D = 64
H = 4
DH = 16
P = 128
NCORES = 8
TPC = NP // P // NCORES      # 98 tiles per core
OWN = TPC * P                # 12544 nodes per core
EXP_BIAS = float(np.log(2.0 ** -8))   # scale exp by 1/256: fp16-safe, cancels in ratio

_COMPILED = {}
TRACE = [False]      # test harness can enable NTFF tracing
LAST = [None]        # last BassKernelResults (for exec_time_ns)


def _host_prep(emb, Wq, Wk, Wv, edge_index):
    """Degree-sort nodes, build per-core slot grids + transposed gathered
    source embeddings."""
    rows = edge_index[0].astype(np.int64)
    cols = edge_index[1].astype(np.int64)

    deg = np.bincount(rows, minlength=NP)
    order = np.argsort(-deg, kind='stable')          # rank -> node
    rank_of = np.empty(NP, dtype=np.int64)
    rank_of[order] = np.arange(NP)

    # per-tile chunk capacity, from rank profile (identical across cores;
    # core 0 slot 0 of each tile has the max degree in that tile by sortedness)
    deg_sorted = deg[order]                          # non-increasing
    C = deg_sorted[np.arange(TPC) * (P * NCORES)].astype(np.int64)  # [TPC]
    nz = C > 0
    C_nz = C[nz]
    tiles_nz = np.nonzero(nz)[0]
    tot_slots = int(C_nz.sum()) * P                  # per core

    # slot offsets per tile (slot layout: for tile, for chunk, for partition)
    off = np.zeros(TPC + 1, dtype=np.int64)
    np.cumsum(C * P, out=off[1:])

    # assign each edge to (core, tile, part, chunk)
    r = rank_of[rows]
    core_e = r & 7
    s_local = r >> 3
    tile_e = s_local >> 7
    part_e = s_local & 127
    # chunk = occurrence index of the edge within its destination
    order_e = np.argsort(rows, kind='stable')
    occ = np.zeros(len(rows), dtype=np.int64)
    sorted_rows = rows[order_e]
    starts = np.r_[0, np.cumsum(np.bincount(sorted_rows, minlength=NP))[:-1]]
    occ[order_e] = np.arange(len(rows)) - starts[sorted_rows]

    emb16T = np.zeros((D, NP + 1), dtype=np.float16)
    emb16T[:, :N] = emb.T.astype(np.float16)         # col NP = zeros (pad)

    slot_of_edge = tile_e * 0  # placeholder
    slot_of_edge = off[tile_e] + occ * P + part_e    # within-core slot index

    xcT = np.empty((NCORES, D, tot_slots), dtype=np.float16)
    mask = np.zeros((NCORES, P, int(C_nz.sum()) * H), dtype=np.float16)
    own_nodes = np.empty((NCORES, OWN), dtype=np.int64)
    for c in range(NCORES):
        sel = core_e == c
        colidx = np.full(tot_slots, NP, dtype=np.int64)     # pad -> zero col
        colidx[slot_of_edge[sel]] = cols[sel]
        xcT[c] = emb16T[:, colidx]
        # mask[p, (cum_chunk c)*4 + h] = 1 if slot valid
        m = np.zeros((tot_slots // P, P), dtype=np.float16)  # [chunkglobal, part]
        valid = np.zeros(tot_slots, dtype=bool)
        valid[slot_of_edge[sel]] = True
        m[:] = valid.reshape(-1, P)
        # expand to heads: mask layout per tile: [128, C*4] with (c,h) c-major
        mask[c] = np.repeat(m.T, H, axis=1)          # [P, chunks*4]
        own_nodes[c] = order[c::8]

    embT_own = np.empty((NCORES, D, OWN), dtype=np.float16)
    for c in range(NCORES):
        embT_own[c] = emb16T[:, own_nodes[c]]

    w3 = np.concatenate([Wq, Wk, Wv], axis=1).astype(np.float16)  # [64, 192]
    return dict(xcT=xcT, mask=mask, embT_own=embT_own, w3=w3,
                C_nz=tuple(int(x) for x in C_nz), tiles_nz=tiles_nz,
                order=order, tot_slots=tot_slots)


def _build_program(C_nz, tot_slots):
    """Build the SPMD Bass program for one core. C_nz: chunk count per
    non-empty tile."""
    import bassboot  # noqa: F401  (env fixups; safe if already imported)
    import concourse.bass as bass
    import concourse.mybir as mybir
    import concourse.tile as tile

    f16, f32 = mybir.dt.float16, mybir.dt.float32
    nt = len(C_nz)
    nchunks = sum(C_nz)

    nc = bass.Bass()
    xcT_d = nc.declare_dram_parameter("xcT", [D, tot_slots], f16, isOutput=False)
    mask_d = nc.declare_dram_parameter("mask", [P, nchunks * H], f16, isOutput=False)
    eTo_d = nc.declare_dram_parameter("embT_own", [D, OWN], f16, isOutput=False)
    w3_d = nc.declare_dram_parameter("w3", [D, 3 * D], f16, isOutput=False)
    out_d = nc.declare_dram_parameter("out", [OWN, D], f32, isOutput=True)

    with tile.TileContext(nc) as tc:
        with tc.tile_pool(name="const", bufs=1) as cpool, \
             tc.tile_pool(name="sb", bufs=2) as sb, \
             tc.tile_pool(name="slab", bufs=2) as slab_pool, \
             tc.tile_pool(name="ps", bufs=2, space="PSUM") as ps:
            w3_sb = cpool.tile([D, 3 * D], f16)
            nc.sync.dma_start(out=w3_sb[:], in_=w3_d[:, :])
            eTo_sb = cpool.tile([D, OWN], f16)        # 24.5KB/part on 64 parts
            nc.sync.dma_start(out=eTo_sb[:], in_=eTo_d[:, :])
            mask_sb = cpool.tile([P, nchunks * H], f16)
            nc.sync.dma_start(out=mask_sb[:], in_=mask_d[:, :])
            bias_t = cpool.tile([P, 1], f32)
            nc.vector.memset(bias_t[:], EXP_BIAS)

            # group tiles into DMA slabs of >=1MB (>= 4096 slot-columns)
            groups = []
            g = []
            gcols = 0
            for i in range(nt):
                g.append(i)
                gcols += C_nz[i] * P
                if gcols >= 4096:
                    groups.append(g)
                    g, gcols = [], 0
            if g:
                groups.append(g)

            slot0 = {}
            acc = 0
            for i in range(nt):
                slot0[i] = acc
                acc += C_nz[i] * P

            for g in groups:
                g0, g1 = slot0[g[0]], slot0[g[-1]] + C_nz[g[-1]] * P
                xc_sl = slab_pool.tile([D, g1 - g0], f16, tag="xc")
                nc.sync.dma_start(out=xc_sl[:], in_=xcT_d[:, g0:g1])
                for ti in g:
                    C = C_nz[ti]
                    base = slot0[ti] - g0
                    ch0 = slot0[ti] // P          # global chunk offset
                    # q for this tile: [128n, 64] psum
                    q_ps = ps.tile([P, D], f32, tag="q")
                    nc.tensor.matmul(q_ps[:], lhsT=eTo_sb[:, ti * P:(ti + 1) * P],
                                     rhs=w3_sb[:, 0:D], start=True, stop=True)
                    # qC: replicate q across chunks, cast fp16 (ACT, PSUM src)
                    qC = sb.tile([P, C * D], f16, tag="qC")
                    nc.scalar.activation(
                        out=qC[:].rearrange("p (c d) -> p c d", d=D),
                        in_=q_ps[:, None, :].to_broadcast([P, C, D]),
                        func=mybir.ActivationFunctionType.Copy)
                    # k|v per chunk; batch 4 chunks per PSUM bank
                    kv = sb.tile([P, C * P], f16, tag="kv")
                    for b0 in range(0, C, 4):
                        bn = min(4, C - b0)
                        kv_ps = ps.tile([P, 512], f32, tag="kvps")
                        for j in range(bn):
                            c = b0 + j
                            nc.tensor.matmul(
                                kv_ps[:, j * P:(j + 1) * P],
                                lhsT=xc_sl[:, base + c * P: base + (c + 1) * P],
                                rhs=w3_sb[:, D:3 * D], start=True, stop=True)
                        nc.scalar.activation(
                            out=kv[:, b0 * P:(b0 + bn) * P],
                            in_=kv_ps[:, 0:bn * P],
                            func=mybir.ActivationFunctionType.Copy)
                    kv3 = kv[:].rearrange("p (c e) -> p c e", e=P)
                    # p = qC * k
                    pm = sb.tile([P, C * D], f16, tag="pm")
                    nc.vector.tensor_mul(
                        out=pm[:].rearrange("p (c d) -> p c d", d=D),
                        in0=qC[:].rearrange("p (c d) -> p c d", d=D),
                        in1=kv3[:, :, 0:D])
                    # att = head-sums -> f32 [128, C*4]
                    att = sb.tile([P, C * H], f32, tag="att")
                    nc.vector.reduce_sum(
                        out=att[:].rearrange("p (c h) -> p c h", h=H),
                        in_=pm[:].rearrange("p (c h d) -> p c h d", h=H, d=DH),
                        axis=mybir.AxisListType.X)
                    # clip to [-10, 10]
                    nc.vector.tensor_scalar(
                        out=att[:], in0=att[:],
                        scalar1=10.0, scalar2=-10.0,
                        op0=mybir.AluOpType.min, op1=mybir.AluOpType.max)
                    # exp (scaled) -> fp16, then mask invalid slots to 0
                    expm = sb.tile([P, C * H], f16, tag="expm")
                    nc.scalar.activation(out=expm[:], in_=att[:],
                                         func=mybir.ActivationFunctionType.Exp,
                                         bias=bias_t[:])
                    nc.vector.tensor_mul(
                        out=expm[:], in0=expm[:],
                        in1=mask_sb[:, ch0 * H:(ch0 + C) * H])
                    # eR: replicate exp over dh (ACT)
                    eR = sb.tile([P, C * D], f16, tag="eR")
                    nc.scalar.activation(
                        out=eR[:].rearrange("p (c h d) -> p c h d", h=H, d=DH),
                        in_=expm[:].rearrange("p (c h) -> p c h", h=H)[:, :, :, None]
                            .to_broadcast([P, C, H, DH]),
                        func=mybir.ActivationFunctionType.Copy)
                    # num = eR * v
                    num = sb.tile([P, C * D], f16, tag="num")
                    nc.vector.tensor_mul(
                        out=num[:].rearrange("p (c d) -> p c d", d=D),
                        in0=eR[:].rearrange("p (c d) -> p c d", d=D),
                        in1=kv3[:, :, D:2 * D])
                    # segment sums over chunk axis
                    accn = sb.tile([P, D], f32, tag="accn")
                    nc.vector.reduce_sum(
                        out=accn[:],
                        in_=num[:].rearrange("p (c d) -> p d c", d=D),
                        axis=mybir.AxisListType.X)
                    accd = sb.tile([P, H], f32, tag="accd")
                    nc.vector.reduce_sum(
                        out=accd[:],
                        in_=expm[:].rearrange("p (c h) -> p h c", h=H),
                        axis=mybir.AxisListType.X)
                    # out = accn / (accd + eps)
                    nc.vector.tensor_scalar_add(
                        out=accd[:], in0=accd[:],
                        scalar1=1e-8 * (2.0 ** -8))
                    rden = sb.tile([P, H], f32, tag="rden")
                    nc.vector.reciprocal(out=rden[:], in_=accd[:])
                    outt = sb.tile([P, D], f32, tag="outt")
                    nc.vector.tensor_mul(
                        out=outt[:].rearrange("p (h d) -> p h d", d=DH),
                        in0=accn[:].rearrange("p (h d) -> p h d", d=DH),
                        in1=rden[:, :, None].to_broadcast([P, H, DH]))
                    nc.sync.dma_start(out=out_d[ti * P:(ti + 1) * P, :], in_=outt[:])
    return nc


def kernel(all_embeddings, Wq, Wk, Wv, edge_index):
    import bassboot  # noqa: F401
    from concourse.bass_utils import run_bass_kernel_spmd

    emb = np.asarray(all_embeddings, dtype=np.float32)
    Wq = np.asarray(Wq, dtype=np.float32)
    Wk = np.asarray(Wk, dtype=np.float32)
    Wv = np.asarray(Wv, dtype=np.float32)
    ei = np.asarray(edge_index)

    prep = _host_prep(emb, Wq, Wk, Wv, ei)
    key = (prep['C_nz'], prep['tot_slots'])
    if key not in _COMPILED:
        _COMPILED[key] = _build_program(list(prep['C_nz']), prep['tot_slots'])
    nc = _COMPILED[key]

    in_maps = []
    for c in range(NCORES):
        in_maps.append({
            "xcT": np.ascontiguousarray(prep['xcT'][c]),
            "mask": np.ascontiguousarray(prep['mask'][c]),
            "embT_own": np.ascontiguousarray(prep['embT_own'][c]),
            "w3": prep['w3'],
        })
    res = run_bass_kernel_spmd(nc, in_maps, core_ids=list(range(NCORES)),
                               trace=TRACE[0])
    LAST[0] = res

    order = prep['order']
    tiles_nz = set(int(t) for t in prep['tiles_nz'])
    out = np.zeros((NP, D), dtype=np.float32)
    for c in range(NCORES):
        oc = res.results[c]["out"]                   # [OWN, 64]
        # zero rows of skipped (deg-0) tiles
        for ti in range(TPC):
            if ti not in tiles_nz:
                oc[ti * P:(ti + 1) * P] = 0.0
        out[order[c::8]] = oc
    return out[:N]
